# revision 3
# baseline (speedup 1.0000x reference)
# Trainium2 Bass kernel for nn_Bridge_BlockV1 (dense_mlp, compute regime).
#
# Strategy: data-parallel over batch across 8 NeuronCores. All layout work is
# done on the host so the device kernel is pure streaming with zero on-chip
# transposes:
#   * features are permuted j-major (feat' = j*256 + m) and activations are
#     passed transposed: XrT/XiT [4096, B] (feature on partitions).
#   * r_W is passed as W' = [in_feat', out_feat'] so big-GEMM stationaries
#     load with the contraction dim on partitions natively.
#   * the per-batch [16,256]@[256,256] c_W GEMM collapses (the swapaxes pair
#     cancels into a left-multiplication by c_W) into contiguous-partition
#     matmuls over the j-major blocks.
#   * all affine constants (0.5 ln scale, weight_lam/tha through c_W, c_b,
#     bias_lam/tha, r_b) are folded on the host into the stationary matrices
#     and per-partition bias vectors.
# Matmuls run in float32r (full fp32 data, TF32-class rounding, 1 cycle/row).
# cos(x) = sin(x + pi/2); angles are range-reduced with two conditional
# +/-2pi wraps (exact for |T| <= 5pi; actual range here is |T| <= ~3.4).
import sys

sys.path.insert(0, "/opt/trn_rl_repo")

import numpy as np

N_CORES = 8
B = 8192
F = 4096
BC = B // N_CORES          # 1024 batch per core
NCH = 2                    # b-chunks per core
CH = BC // NCH             # 512 = moving free dim
KT = F // 128              # 32 k chunks
NT = F // 128              # 32 out tiles
PI = float(np.pi)
TWO_PI = float(2 * np.pi)

_cache = {}


def _build_program():
    import concourse.bass as bass
    import concourse.tile as tile
    from concourse import bacc, mybir

    F32 = mybir.dt.float32
    F32R = mybir.dt.float32r
    AF = mybir.ActivationFunctionType
    ALU = mybir.AluOpType

    nc = bacc.Bacc(None, target_bir_lowering=False, debug=False, num_devices=N_CORES)

    xr_d = nc.dram_tensor("xr", [KT, 128, BC], F32R, kind="ExternalInput").ap()
    xi_d = nc.dram_tensor("xi", [KT, 128, BC], F32R, kind="ExternalInput").ap()
    wp_d = nc.dram_tensor("wp", [F, F], F32R, kind="ExternalInput").ap()
    cws_d = nc.dram_tensor("cws", [8, 128, 128], F32R, kind="ExternalInput").ap()
    bexp_d = nc.dram_tensor("bexp", [128, NT], F32, kind="ExternalInput").ap()
    bcos_d = nc.dram_tensor("bcos", [128, NT], F32, kind="ExternalInput").ap()
    bsin_d = nc.dram_tensor("bsin", [128, NT], F32, kind="ExternalInput").ap()
    rbp_d = nc.dram_tensor("rbp", [128, NT], F32, kind="ExternalInput").ap()
    rt_d = nc.dram_tensor("rt", [F, BC], F32, kind="ExternalOutput").ap()
    it_d = nc.dram_tensor("it", [F, BC], F32, kind="ExternalOutput").ap()

    xr_r = xr_d.rearrange("ft p b -> p ft b")
    xi_r = xi_d.rearrange("ft p b -> p ft b")
    wp_r = wp_d.rearrange("(kc p) (nt c) -> p kc nt c", p=128, c=128)
    cws_r = cws_d.rearrange("s p c -> p s c")
    rt_r = rt_d.rearrange("(nt p) b -> nt p b", p=128)
    it_r = it_d.rearrange("(nt p) b -> nt p b", p=128)

    with tile.TileContext(nc) as tc:
        with (
            tc.tile_pool(name="xpool", bufs=1) as xpool,
            tc.tile_pool(name="wpool", bufs=3) as wpool,
            tc.tile_pool(name="cpool", bufs=1) as cpool,
            tc.tile_pool(name="br", bufs=1) as br,
            tc.tile_pool(name="br2", bufs=1) as br2,
            tc.tile_pool(name="tr", bufs=1) as tr,
            tc.tile_pool(name="wy", bufs=2) as wyp,
            tc.tile_pool(name="ep", bufs=1) as ep,
            tc.tile_pool(name="pbig", bufs=2, space="PSUM") as pbig,
            tc.tile_pool(name="psml", bufs=1, space="PSUM") as psml,
        ):
            cwt = cpool.tile([128, 8, 128], F32R, tag="cws")
            nc.sync.dma_start(cwt[:], cws_r[:])
            bexp_t = cpool.tile([128, NT], F32, tag="bexp")
            nc.sync.dma_start(bexp_t[:], bexp_d[:])
            bcos_t = cpool.tile([128, NT], F32, tag="bcos")
            nc.sync.dma_start(bcos_t[:], bcos_d[:])
            bsin_t = cpool.tile([128, NT], F32, tag="bsin")
            nc.sync.dma_start(bsin_t[:], bsin_d[:])
            rbp_t = cpool.tile([128, NT], F32, tag="rbp")
            nc.sync.dma_start(rbp_t[:], rbp_d[:])
            eps2 = cpool.tile([128, 1], F32, tag="eps2")
            nc.vector.memset(eps2[:], 2e-6)

            for bc in range(NCH):
                bsl = bass.ds(bc * CH, CH)
                xr_c = xpool.tile([128, KT, CH], F32R, tag="xr")
                nc.sync.dma_start(xr_c[:], xr_r[:, :, bsl])
                xi_c = xpool.tile([128, KT, CH], F32R, tag="xi")
                nc.sync.dma_start(xi_c[:], xi_r[:, :, bsl])

                for j in range(16):
                    # ---- l/t branch (per m-half to keep SBUF small) ----
                    lnm = br2.tile([128, 2, CH], F32R, tag="lnm")
                    tmid = br2.tile([128, 2, CH], F32R, tag="tmid")
                    for mh in range(2):
                        ft = 2 * j + mh
                        xv = xr_c[:, ft, :].bitcast(F32)
                        yv = xi_c[:, ft, :].bitcast(F32)
                        sqr = br.tile([128, CH], F32, tag="sqr")
                        nc.scalar.activation(sqr[:], xv, AF.Square)
                        sqi = br.tile([128, CH], F32, tag="sqi")
                        nc.scalar.activation(sqi[:], yv, AF.Square)
                        lmid = br.tile([128, CH], F32, tag="lmid")
                        nc.vector.tensor_tensor(lmid[:], sqr[:], sqi[:], ALU.add)
                        nc.scalar.activation(lnm[:, mh, :], lmid[:], AF.Ln, bias=eps2[:, :])

                        xp = br.tile([128, CH], F32, tag="xp")
                        nc.vector.tensor_scalar_add(xp[:], xv, 1e-6)
                        yp = br.tile([128, CH], F32, tag="yp")
                        nc.vector.tensor_scalar_add(yp[:], yv, 1e-6)
                        rec = br.tile([128, CH], F32, tag="rec")
                        nc.vector.reciprocal(rec[:], xp[:])
                        q = br.tile([128, CH], F32, tag="q")
                        nc.vector.tensor_tensor(q[:], yp[:], rec[:], ALU.mult)
                        at = br.tile([128, CH], F32, tag="at")
                        nc.scalar.activation(at[:], q[:], AF.Arctan)
                        sg = br.tile([128, CH], F32, tag="sg")
                        nc.scalar.activation(sg[:], yp[:], AF.Sign)
                        msk = br.tile([128, CH], F32, tag="sqr")
                        nc.vector.tensor_scalar(msk[:], xp[:], 0.0, None, ALU.is_lt)
                        corr = br.tile([128, CH], F32, tag="sqi")
                        nc.vector.tensor_tensor(corr[:], msk[:], sg[:], ALU.mult)
                        nc.vector.scalar_tensor_tensor(
                            tmid[:, mh, :], corr[:], PI, at[:], ALU.mult, ALU.add
                        )

                    # ---- small GEMMs: lout/tout for both kh ----
                    psl = psml.tile([128, 2, CH], F32, tag="pl")
                    pst = psml.tile([128, 2, CH], F32, tag="pt")
                    for kh in range(2):
                        for mh in range(2):
                            nc.tensor.matmul(
                                psl[:, kh, :], cwt[:, 0 * 4 + mh * 2 + kh, :],
                                lnm[:, mh, :], start=(mh == 0), stop=(mh == 1),
                            )
                        for mh in range(2):
                            nc.tensor.matmul(
                                pst[:, kh, :], cwt[:, 1 * 4 + mh * 2 + kh, :],
                                tmid[:, mh, :], start=(mh == 0), stop=(mh == 1),
                            )

                    # ---- trig / exp ----
                    lfin = tr.tile([128, 2, CH], F32, tag="lfin")
                    for kh in range(2):
                        nt = 2 * j + kh
                        nc.scalar.activation(
                            lfin[:, kh, :], psl[:, kh, :], AF.Exp,
                            bias=bexp_t[:, nt : nt + 1],
                        )

                    def reduced_sin(bias_t, out_tag):
                        xb = tr.tile([128, 2, CH], F32, tag="xb")
                        for kh in range(2):
                            nt = 2 * j + kh
                            nc.vector.tensor_scalar(
                                xb[:, kh, :], pst[:, kh, :],
                                bias_t[:, nt : nt + 1], None, ALU.add,
                            )
                        m1 = br.tile([128, 2, CH], F32, tag="wm")
                        nc.vector.tensor_scalar(m1[:], xb[:], PI, None, ALU.is_gt)
                        y1 = wyp.tile([128, 2, CH], F32, tag="wy")
                        nc.vector.scalar_tensor_tensor(y1[:], m1[:], -TWO_PI, xb[:], ALU.mult, ALU.add)
                        m2 = br.tile([128, 2, CH], F32, tag="wm")
                        nc.vector.tensor_scalar(m2[:], y1[:], -PI, None, ALU.is_lt)
                        y2 = wyp.tile([128, 2, CH], F32, tag="wy")
                        nc.vector.scalar_tensor_tensor(y2[:], m2[:], TWO_PI, y1[:], ALU.mult, ALU.add)
                        m3 = br.tile([128, 2, CH], F32, tag="wm")
                        nc.vector.tensor_scalar(m3[:], y2[:], PI, None, ALU.is_gt)
                        y3 = wyp.tile([128, 2, CH], F32, tag="wy")
                        nc.vector.scalar_tensor_tensor(y3[:], m3[:], -TWO_PI, y2[:], ALU.mult, ALU.add)
                        m4 = br.tile([128, 2, CH], F32, tag="wm")
                        nc.vector.tensor_scalar(m4[:], y3[:], -PI, None, ALU.is_lt)
                        y4 = wyp.tile([128, 2, CH], F32, tag="wy")
                        nc.vector.scalar_tensor_tensor(y4[:], m4[:], TWO_PI, y3[:], ALU.mult, ALU.add)
                        out = tr.tile([128, 2, CH], F32, tag=out_tag)
                        nc.scalar.activation(out[:], y4[:], AF.Sin)
                        return out

                    cs = reduced_sin(bcos_t, "cs")
                    sn = reduced_sin(bsin_t, "sn")

                    # ---- big GEMMs + epilogue per kh ----
                    for kh in range(2):
                        nt = 2 * j + kh
                        wts = []
                        for wq in range(4):
                            wt_ = wpool.tile([128, 8, 128], F32R, tag="wt")
                            nc.sync.dma_start(wt_[:], wp_r[:, 8 * wq : 8 * (wq + 1), nt, :])
                            wts.append(wt_)
                        pr = pbig.tile([128, CH], F32, tag="pr")
                        pi_ = pbig.tile([128, CH], F32, tag="pi")
                        for kc in range(KT):
                            wv = wts[kc // 8][:, kc % 8, :]
                            nc.tensor.matmul(pr[:], wv, xr_c[:, kc, :],
                                             start=(kc == 0), stop=(kc == KT - 1))
                            nc.tensor.matmul(pi_[:], wv, xi_c[:, kc, :],
                                             start=(kc == 0), stop=(kc == KT - 1))

                        lc = ep.tile([128, CH], F32, tag="lc")
                        nc.vector.tensor_tensor(lc[:], lfin[:, kh, :], cs[:, kh, :], ALU.mult)
                        sr = ep.tile([128, CH], F32, tag="sr")
                        nc.vector.scalar_tensor_tensor(
                            sr[:], lc[:], rbp_t[:, nt : nt + 1], pr[:], ALU.add, ALU.add
                        )
                        nc.sync.dma_start(rt_r[nt, :, bsl], sr[:])

                        li = ep.tile([128, CH], F32, tag="li")
                        nc.vector.tensor_tensor(li[:], lfin[:, kh, :], sn[:, kh, :], ALU.mult)
                        si = ep.tile([128, CH], F32, tag="si")
                        nc.vector.scalar_tensor_tensor(
                            si[:], li[:], rbp_t[:, nt : nt + 1], pi_[:], ALU.add, ALU.add
                        )
                        nc.sync.dma_start(it_r[nt, :, bsl], si[:])

    nc.compile()
    return nc


def _get_runner():
    if "runner" in _cache:
        return _cache["runner"]
    import jax
    from jax.sharding import Mesh, NamedSharding, PartitionSpec
    from jax.experimental.shard_map import shard_map
    from concourse import mybir
    from concourse.bass2jax import _bass_exec_p, install_neuronx_cc_hook, partition_id_tensor

    nc = _build_program()
    install_neuronx_cc_hook()
    partition_name = nc.partition_id_tensor.name if nc.partition_id_tensor else None
    in_names, out_names, out_avals = [], [], []
    for alloc in nc.m.functions[0].allocations:
        if not isinstance(alloc, mybir.MemoryLocationSet):
            continue
        name = alloc.memorylocations[0].name
        if alloc.kind == "ExternalInput":
            if name != partition_name:
                in_names.append(name)
        elif alloc.kind == "ExternalOutput":
            out_names.append(name)
            out_avals.append(
                jax.core.ShapedArray(tuple(alloc.tensor_shape), mybir.dt.np(alloc.dtype))
            )
    all_names = list(in_names) + list(out_names)
    if partition_name is not None:
        all_names.append(partition_name)

    def _body(*args):
        operands = list(args)
        if partition_name is not None:
            operands.append(partition_id_tensor())
        return tuple(
            _bass_exec_p.bind(
                *operands,
                out_avals=tuple(out_avals),
                in_names=tuple(all_names),
                out_names=tuple(out_names),
                lowering_input_output_aliases=(),
                sim_require_finite=True,
                sim_require_nnan=True,
                nc=nc,
            )
        )

    devices = jax.devices()[:N_CORES]
    mesh = Mesh(np.asarray(devices), ("core",))
    n_params = len(in_names)
    n_outs = len(out_names)
    fn = jax.jit(
        shard_map(
            _body,
            mesh=mesh,
            in_specs=(PartitionSpec("core"),) * (n_params + n_outs),
            out_specs=(PartitionSpec("core"),) * n_outs,
            check_rep=False,
        ),
        keep_unused=True,
    )
    runner = {
        "fn": fn,
        "mesh": mesh,
        "in_names": in_names,
        "out_names": out_names,
        "out_avals": out_avals,
        "NamedSharding": NamedSharding,
        "PartitionSpec": PartitionSpec,
        "jax": jax,
    }
    _cache["runner"] = runner
    return runner


def _host_pack(f_r, f_i, r_W, r_b, c_W, c_b, weight_lam, weight_tha, bias_lam, bias_tha):
    f_r = np.asarray(f_r, np.float32)
    f_i = np.asarray(f_i, np.float32)
    r_W = np.asarray(r_W, np.float32)
    r_b = np.asarray(r_b, np.float32)
    c_W = np.asarray(c_W, np.float32)
    c_b = np.asarray(c_b, np.float32)
    wlam = np.asarray(weight_lam, np.float32)[0]
    wtha = np.asarray(weight_tha, np.float32)[0]
    blam = np.asarray(bias_lam, np.float32)[0]
    btha = np.asarray(bias_tha, np.float32)[0]

    XrT = np.ascontiguousarray(f_r.transpose(2, 1, 0).reshape(KT, 128, B))
    XiT = np.ascontiguousarray(f_i.transpose(2, 1, 0).reshape(KT, 128, B))
    W4 = r_W.reshape(256, 16, 256, 16)
    Wp = np.ascontiguousarray(W4.transpose(3, 2, 1, 0).reshape(F, F))

    cwt_l = 0.5 * c_W.T
    cwt_t = np.ascontiguousarray(c_W.T)
    cws = np.empty((8, 128, 128), np.float32)
    for lt, base in ((0, cwt_l), (1, cwt_t)):
        for mh in range(2):
            for kh in range(2):
                cws[lt * 4 + mh * 2 + kh] = base[
                    mh * 128 : (mh + 1) * 128, kh * 128 : (kh + 1) * 128
                ]

    bias_l = (c_b[None, :] + blam + (c_W @ wlam).T).astype(np.float32).reshape(F)
    bias_t = (c_b[None, :] + btha + (c_W @ wtha).T).astype(np.float32).reshape(F)
    rbp = r_b.reshape(256, 16).T.reshape(F)

    def pack(v):
        return np.ascontiguousarray(v.reshape(NT, 128).T.astype(np.float32))

    common = {
        "wp": Wp,
        "cws": cws,
        "bexp": pack(bias_l),
        "bcos": pack(bias_t + np.float32(np.pi / 2)),
        "bsin": pack(bias_t),
        "rbp": pack(rbp),
    }
    in_maps = []
    for c in range(N_CORES):
        sl = slice(c * BC, (c + 1) * BC)
        m = dict(common)
        m["xr"] = np.ascontiguousarray(XrT[:, :, sl])
        m["xi"] = np.ascontiguousarray(XiT[:, :, sl])
        in_maps.append(m)
    return in_maps


def _run(in_maps):
    r = _get_runner()
    jax = r["jax"]
    NamedSharding, PartitionSpec = r["NamedSharding"], r["PartitionSpec"]
    sh = NamedSharding(r["mesh"], PartitionSpec("core"))
    args = []
    for name in r["in_names"]:
        concat = np.concatenate([m[name] for m in in_maps], axis=0)
        args.append(jax.device_put(concat, sh))
    for av in r["out_avals"]:
        z = np.zeros((N_CORES * av.shape[0], *av.shape[1:]), av.dtype)
        args.append(jax.device_put(z, sh))
    outs = r["fn"](*args)
    jax.block_until_ready(outs)
    res = {}
    for i, name in enumerate(r["out_names"]):
        res[name] = np.asarray(outs[i])  # [N_CORES*F, BC]
    return res


def kernel(**inputs):
    in_maps = _host_pack(**inputs)
    res = _run(in_maps)
    rt = res["rt"].reshape(N_CORES, F, BC)
    it = res["it"].reshape(N_CORES, F, BC)
    RT = np.concatenate([rt[c] for c in range(N_CORES)], axis=1)  # [F, B]
    IT = np.concatenate([it[c] for c in range(N_CORES)], axis=1)
    r = np.ascontiguousarray(RT.reshape(16, 256, B).transpose(2, 1, 0))
    i = np.ascontiguousarray(IT.reshape(16, 256, B).transpose(2, 1, 0))
    return (r, i)
